# revision 15
# baseline (speedup 1.0000x reference)
"""Multi-head causal attention (B=2, T=2048, D=2048, H=16) on 8 trn2 NeuronCores.

Sharding: tensor-parallel over heads (2 heads/core); x replicated, W_qkv
column-sliced, W_out row-sliced; host sums the 8 full-shape y partials.

Design (v2, software-pipelined; ~418us/iter HW vs 574us baseline):
- Per 512-token window: QKV projection (stage1) -> causal attention rows
  (stage2, j == tau) -> output projection, with the out-projection of window
  N woven between the attention steps of window N+1 so the PE never starves
  and the y DMA spreads across the whole kernel.
- V computed in natural [tok, feat] layout via x-stationary matmuls (no PE
  transposes); V bias folded into b_out on the host (softmax weights sum
  to 1, so the bias commutes through the normalized attention).
- bf16 everywhere (x, weights, q/k/v, exp'd scores, y): same PE rate as
  f32r at >=256 free-size, half the DMA/SBUF, 2x DVE rate. rel err ~7e-3.
- Rowsums via DVE quad pre-adds of exp tiles + one ones-matmul per group
  of 4 (4x fewer PE rows than per-step ones-matmuls); softmax runs without
  max-subtraction (scores bounded) and normalization happens once per
  window via reciprocal -> bf16 ones-broadcast matmul -> DVE multiply.
- PSUM pooled by lifetime class (4+2+2 banks = exactly 16KB/partition);
  exp->P@V lag of 2 steps hides ACT latency.

Measured dead ends (regress on HW despite cost-model wins): fine-grained
diagonal trimming via sub-range PSUM APs (+33..80us), fp8 DoubleRow q/k
projections (fails the 2e-2 gate at 4.7e-2: score noise passes ~1:1 into
output error), GPSIMD for RoPE copies or causal masking (+12us).
"""

import math
import os

import numpy as np

import concourse.bass as bass
import concourse.mybir as mybir
import concourse.tile as tile
from concourse import bacc
from concourse.bass_utils import run_bass_kernel_spmd

B, T, D_IN, D_MODEL, H = 2, 2048, 2048, 2048, 16
DH = 128
NCORES = 8
HPC = H // NCORES  # heads per core (2)
BT = B * T
SCALE = 1.0 / math.sqrt(DH)

F32 = mybir.dt.float32
F32R = mybir.dt.float32r
BF16 = mybir.dt.bfloat16
AF = mybir.ActivationFunctionType
ALU = mybir.AluOpType

TOKT = 512             # token window
NTT = T // TOKT        # windows per batch (4)
NDCH = D_IN // 128     # d_in contraction chunks (16)
NFT = D_MODEL // 512   # output feature tiles (4)


def build_nc(debug=False, reps=1):
    nc = bacc.Bacc("TRN2", target_bir_lowering=False, debug=False,
                   num_devices=NCORES)

    xT = nc.dram_tensor("xT", [D_IN, BT], BF16, kind="ExternalInput")
    wq = nc.dram_tensor("wq", [D_IN, HPC * DH], BF16, kind="ExternalInput")
    wk = nc.dram_tensor("wk", [D_IN, HPC * DH], BF16, kind="ExternalInput")
    wv = nc.dram_tensor("wv", [D_IN, HPC * DH], BF16, kind="ExternalInput")
    bq = nc.dram_tensor("bq", [HPC * DH], F32, kind="ExternalInput")
    bk = nc.dram_tensor("bk", [HPC * DH], F32, kind="ExternalInput")
    wo = nc.dram_tensor("wo", [HPC * DH, D_MODEL], BF16, kind="ExternalInput")
    cosT = nc.dram_tensor("cosT", [DH, T], F32, kind="ExternalInput")
    sinTs = nc.dram_tensor("sinTs", [DH, T], F32, kind="ExternalInput")
    y = nc.dram_tensor("y", [BT, D_MODEL], BF16, kind="ExternalOutput")

    with tile.TileContext(nc) as tc:
        with (
            tc.tile_pool(name="persist", bufs=1) as pp,
            tc.tile_pool(name="weights", bufs=1) as wp,
            tc.tile_pool(name="qkv", bufs=1) as qp,
        ):
            # ---- per-core weights (first quarter first, to unblock tau 0)
            wq_sb = wp.tile([128, NDCH, HPC * DH], BF16, name="wq_sb")
            wk_sb = wp.tile([128, NDCH, HPC * DH], BF16, name="wk_sb")
            wv_sb = wp.tile([128, NDCH, HPC * DH], BF16, name="wv_sb")

            def _w_quarter(hf):
                for t_, d_ in ((wq_sb, wq), (wk_sb, wk), (wv_sb, wv)):
                    nc.sync.dma_start(
                        t_[:, hf * (NDCH // 4):(hf + 1) * (NDCH // 4), :],
                        d_.ap()[hf * (D_IN // 4):(hf + 1) * (D_IN // 4), :]
                        .rearrange("(c p) f -> p c f", p=128))

            _w_quarter(0)

            xs_cm = tc.tile_pool(name="xs", bufs=9)
            xs = xs_cm.__enter__()

            # ---- constants
            cos_f = pp.tile([DH, T], F32, name="cos_f")
            sin_f = pp.tile([DH, T], F32, name="sin_f")
            nc.sync.dma_start(cos_f[:], cosT.ap())
            nc.sync.dma_start(sin_f[:], sinTs.ap())
            cos_sb = pp.tile([DH, T], BF16, name="cos_sb")
            sin_sb = pp.tile([DH, T], BF16, name="sin_sb")
            nc.scalar.copy(cos_sb[:], cos_f[:])
            nc.scalar.copy(sin_sb[:], sin_f[:])
            ones1 = pp.tile([1, 128], BF16, name="ones1")
            nc.gpsimd.memset(ones1[:], 1.0)
            onescol = pp.tile([128, 1], BF16, name="onescol")
            nc.gpsimd.memset(onescol[:], 1.0)
            bqt = pp.tile([128, HPC], F32, name="bqt")
            bkt = pp.tile([128, HPC], F32, name="bkt")
            nc.sync.dma_start(bqt[:], bq.ap().rearrange("(h d) -> d h", d=DH))
            nc.sync.dma_start(bkt[:], bk.ap().rearrange("(h d) -> d h", d=DH))

            wo_sb = wp.tile([128, HPC, D_MODEL], BF16, name="wo_sb")
            nc.sync.dma_start(wo_sb[:],
                              wo.ap().rearrange("(h p) f -> p h f", p=128))
            for hf in range(1, 4):
                _w_quarter(hf)

            # ---- per-batch Q^T/K^T/V buffers (single-buffered: batches
            # don't overlap except trailing emit, which reads only ot/wo)
            qT_sb = [qp.tile([DH, T], BF16, name=f"qT{h}") for h in range(HPC)]
            kT_sb = [qp.tile([DH, T], BF16, name=f"kT{h}") for h in range(HPC)]
            v_sb = qp.tile([128, T // 128, HPC * DH], BF16, name="v_sb")

            import contextlib
            rep_ctx = (tc.For_i(0, reps, 1, hint_engines=(
                mybir.EngineType.PE, mybir.EngineType.Activation,
                mybir.EngineType.DVE, mybir.EngineType.Pool,
                mybir.EngineType.SP))
                if reps > 1 else contextlib.nullcontext())
            with rep_ctx:
                _emit_body(nc, tc, xT, wq_sb, wk_sb, wv_sb, bqt, bkt,
                           cos_sb, sin_sb, qT_sb, kT_sb, v_sb, wo_sb, y,
                           ones1, onescol, xs)
            xs_cm.__exit__(None, None, None)
    nc.compile()
    return nc


def _emit_body(nc, tc, xT, wq_sb, wk_sb, wv_sb, bqt, bkt, cos_sb, sin_sb,
               qT_sb, kT_sb, v_sb, wo_sb, y, ones1, onescol, xs):
    """One full forward pass: 8 (b,tau) windows + trailing emit."""
    # PSUM (16KB/partition): big_ps 4x2KB shared by qk-accs/S-tiles/bcast/y,
    # ops 2x2KB (attention O accumulators, live per window), small_ps 2x2KB
    # shared by v-psum (stage1) and rowsum accumulators (window) — disjoint
    # lifetimes, the tile framework serializes reuse hazards.
    with (
        tc.tile_pool(name="st", bufs=4) as st,
        tc.tile_pool(name="big_ps", bufs=4, space="PSUM") as bigps,
        tc.tile_pool(name="ops", bufs=2, space="PSUM") as ops,
        tc.tile_pool(name="small_ps", bufs=2, space="PSUM") as smallps,
        tc.tile_pool(name="pt_p", bufs=12) as ptp,
        tc.tile_pool(name="sum_p", bufs=6) as sump,
        tc.tile_pool(name="ot_p", bufs=2) as otp,
        tc.tile_pool(name="y_p", bufs=4) as yp,
        tc.tile_pool(name="scr", bufs=4) as scr,
    ):
        pend = []  # pending emit work: (b, j, ot_tile)
        evict_rr = [0]  # round-robin for y evictions

        def emit_tiles(work, n):
            """Emit up to n out-projection tiles from the pending list."""
            for _ in range(n):
                if not work:
                    return
                b, j, ot, idx = work[0]
                tt, ft = idx // NFT, idx % NFT
                ps = bigps.tile([128, 512], F32, name="big", tag="big")
                for h in range(HPC):
                    nc.tensor.matmul(
                        ps[:], ot[:, h, tt * 128:(tt + 1) * 128],
                        wo_sb[:, h, ft * 512:(ft + 1) * 512],
                        start=(h == 0), stop=(h == HPC - 1))
                yt = yp.tile([128, 512], BF16, name="y_t")
                # 1:2 ACT:DVE — ACT is the busier engine inside attention
                # windows (it carries all the exp evictions)
                if evict_rr[0] % 3 == 0:
                    nc.scalar.copy(yt[:], ps[:])
                else:
                    nc.vector.tensor_scalar_add(yt[:], ps[:], 0.0)
                evict_rr[0] += 1
                row = b * T + j * TOKT + tt * 128
                nc.sync.dma_start(
                    y.ap()[row:row + 128, ft * 512:(ft + 1) * 512], yt[:])
                if idx + 1 == 16:
                    work.pop(0)
                else:
                    work[0] = (b, j, ot, idx + 1)

        xtiles = {}

        def prefetch(b, tau):
            """Issue the 4 x-quarter DMAs for window (b, tau)."""
            gtok = b * T + tau * TOKT
            tiles = []
            for quarter in range(4):
                xt = xs.tile([128, 4, TOKT], BF16, name="xt")
                nc.sync.dma_start(
                    xt[:],
                    xT.ap()[quarter * 512:(quarter + 1) * 512,
                            gtok:gtok + TOKT]
                    .rearrange("(c p) t -> p c t", p=128))
                tiles.append(xt)
            xtiles[(b, tau)] = tiles

        prefetch(0, 0)
        for b in range(B):
            for tau in range(NTT):
                nxt = (b, tau + 1) if tau + 1 < NTT else (b + 1, 0)
                if nxt[0] < B:
                    prefetch(*nxt)
                _stage1(nc, b, tau, xtiles.pop((b, tau)), wq_sb, wk_sb,
                        wv_sb, bqt, bkt, cos_sb, sin_sb, qT_sb, kT_sb,
                        v_sb, st, bigps, smallps)
                _stage2(nc, b, tau, qT_sb, kT_sb, v_sb, ones1, onescol,
                        bigps, ops, smallps, ptp, sump, otp, scr, pend,
                        emit_tiles)
        emit_tiles(pend, 16)
        assert not pend


def _stage1(nc, b, tau, xts, wq_sb, wk_sb, wv_sb, bqt, bkt, cos_sb, sin_sb,
            qT_sb, kT_sb, v_sb, st, bigps, smallps):
    """QKV projection + RoPE for tokens [tau*512, (tau+1)*512) of batch b.

    Pass 1 (chunk-outer): q/k accumulate in 4 psum banks, w-stationary.
    Pass 2 (block-outer): v in natural [tok, feat] layout, x-stationary.
    """
    pos = tau * TOKT
    accs = [bigps.tile([128, TOKT], F32, name="big", tag="big")
            for _ in range(4)]
    for quarter in range(4):
        xt = xts[quarter]
        for cl in range(4):
            c = quarter * 4 + cl
            for fi, (wsb, hh) in enumerate(
                    ((wq_sb, 0), (wq_sb, 1), (wk_sb, 0), (wk_sb, 1))):
                nc.tensor.matmul(
                    accs[fi][:], wsb[:, c, hh * DH:(hh + 1) * DH],
                    xt[:, cl, :],
                    start=(c == 0), stop=(c == NDCH - 1))
    # q/k evictions with bias (ACT), then RoPE (ACT copies + DVE mults)
    for fi, (bias, dest, hh) in enumerate(
            ((bqt, qT_sb, 0), (bqt, qT_sb, 1),
             (bkt, kT_sb, 0), (bkt, kT_sb, 1))):
        stg = st.tile([128, TOKT], BF16, name="stg")
        nc.scalar.activation(stg[:], accs[fi][:], AF.Identity,
                             bias=bias[:, hh:hh + 1], scale=1.0)
        rot = st.tile([128, TOKT], BF16, name="rot")
        nc.scalar.copy(rot[0:64, :], stg[64:128, :])
        nc.scalar.copy(rot[64:128, :], stg[0:64, :])
        nc.vector.tensor_tensor(
            stg[:], stg[:], cos_sb[:, pos:pos + TOKT], ALU.mult)
        nc.vector.tensor_tensor(
            rot[:], rot[:], sin_sb[:, pos:pos + TOKT], ALU.mult)
        nc.vector.tensor_tensor(
            dest[hh][:, pos:pos + TOKT], stg[:], rot[:], ALU.add)
    # V natural layout: x-stationary, wv moving; no bias (folded into b_out)
    for blk in range(4):
        vp = smallps.tile([128, HPC * DH], F32, name="small", tag="small")
        for quarter in range(4):
            for cl in range(4):
                c = quarter * 4 + cl
                nc.tensor.matmul(
                    vp[:], xts[quarter][:, cl, blk * 128:(blk + 1) * 128],
                    wv_sb[:, c, :],
                    start=(c == 0), stop=(c == NDCH - 1))
        nc.scalar.copy(v_sb[:, tau * 4 + blk, :], vp[:])


def _stage2(nc, b, j, qT_sb, kT_sb, v_sb, ones1, onescol, bigps, ops,
            smallps, ptp, sump, otp, scr, pend, emit_tiles):
    """Causal attention rows q in [j*512, (j+1)*512), both heads, with the
    out-projection of the previous window woven between steps."""
    nstep = 4 * j + 4
    op = [ops.tile([128, 512], F32, name="o_ps") for _ in range(HPC)]
    rp = [smallps.tile([1, 512], F32, name="small", tag="small")
          for _ in range(HPC)]
    pts = [[None] * nstep for _ in range(HPC)]
    sums = [[None] * (j + 1) for _ in range(HPC)]

    # cover the RoPE-drain latency at window start with pending emit work
    emit_tiles(pend, 4)

    def s_step(kk):
        for h in range(HPC):
            sp = bigps.tile([128, 512], F32, name="big", tag="big")
            nc.tensor.matmul(sp[:], kT_sb[h][:, kk * 128:(kk + 1) * 128],
                             qT_sb[h][:, j * 512:(j + 1) * 512],
                             start=True, stop=True)
            pt = ptp.tile([128, 512], BF16, name="pt")
            nc.scalar.activation(pt[:], sp[:], AF.Exp, bias=0.0, scale=SCALE)
            if kk // 4 == j:
                nc.gpsimd.affine_select(
                    out=pt[:], in_=pt[:], compare_op=ALU.is_ge,
                    fill=0.0, base=-(kk % 4) * 128, pattern=[[1, 512]],
                    channel_multiplier=-1)
            pts[h][kk] = pt

    def pv_step(kk):
        for h in range(HPC):
            nc.tensor.matmul(op[h][:],
                             v_sb[:, kk, h * DH:(h + 1) * DH],
                             pts[h][kk], start=(kk == 0),
                             stop=(kk == nstep - 1))
            # rowsum quad pre-adds on DVE (bf16, 2x rate)
            g, r = kk // 4, kk % 4
            if r == 0:
                pass  # group sum starts at r==1
            elif r == 1:
                sm = sump.tile([128, 512], BF16, name="sum")
                nc.vector.tensor_tensor(sm[:], pts[h][kk - 1], pts[h][kk],
                                        ALU.add)
                sums[h][g] = sm
            else:
                nc.vector.tensor_tensor(sums[h][g][:], sums[h][g][:],
                                        pts[h][kk], ALU.add)

    def r_step(g):
        for h in range(HPC):
            nc.tensor.matmul(rp[h][:], onescol[:], sums[h][g][:],
                             start=(g == 0), stop=(g == j))

    # 3-step lag between S/exp and P@V: each cross-engine hop in the
    # S -> exp -> (causal zero) -> P@V chain gets extra slack, which real
    # HW needs more than the cost model suggests (semaphore latencies)
    for s in range(nstep):
        s_step(s)
        if s >= 3:
            pv_step(s - 3)
            if s >= 7 and (s - 7) % 4 == 0:
                r_step((s - 7) // 4)
        emit_tiles(pend, 1)
    for t in (3, 2, 1):
        pv_step(nstep - t)
    r_step(j)

    # normalization: 1/rowsum (DVE) runs under woven emit matmuls, then
    # f32r ones-matmul broadcast, then DVE multiply into ot
    rrinv = [scr.tile([1, 512], BF16, name="rrinv") for _ in range(HPC)]
    with nc.allow_low_precision(reason="1/rowsum in bf16: 0.1% RMS scale "
                                "noise per token, well inside tolerance"):
        for h in range(HPC):
            nc.vector.reciprocal(rrinv[h][:], rp[h][:])
    emit_tiles(pend, 3)
    ot = otp.tile([128, HPC, 512], BF16, name="ot")
    rbs = []
    for h in range(HPC):
        rb = smallps.tile([128, 512], F32, name="small", tag="small")
        nc.tensor.matmul(rb[:], ones1[:], rrinv[h][:],
                         start=True, stop=True)
        rb_sb = scr.tile([128, 512], BF16, name="rb_sb", tag="rb_sb")
        nc.scalar.copy(rb_sb[:], rb[:])
        rbs.append(rb_sb)
    emit_tiles(pend, 1)
    for h in range(HPC):
        nc.vector.tensor_tensor(ot[:, h, :], op[h][:], rbs[h][:], ALU.mult)
    pend.append((b, j, ot, 0))


_CACHE = {}


def _get_nc():
    if "nc" not in _CACHE:
        _CACHE["nc"] = build_nc()
    return _CACHE["nc"]


def _host_prep(x, W_qkv, b_qkv, W_out, mask):
    import ml_dtypes
    bf = ml_dtypes.bfloat16
    xT = np.ascontiguousarray(x.reshape(BT, D_IN).T).astype(bf)
    Wr = W_qkv.reshape(D_IN, H, 3, DH)
    br = b_qkv.reshape(H, 3, DH)
    inv_freq = (1.0 / (10000.0 ** (np.arange(0, DH, 2, dtype=np.float32) / DH))).astype(np.float32)
    tpos = np.arange(T, dtype=np.float32)
    freqs = tpos[:, None] * inv_freq[None, :]
    emb = np.concatenate([freqs, freqs], axis=-1)
    cosT = np.ascontiguousarray(np.cos(emb).astype(np.float32).T)
    sinT = np.sin(emb).astype(np.float32).T
    sinTs = sinT.copy()
    sinTs[0:64] = -sinTs[0:64]
    sinTs = np.ascontiguousarray(sinTs)

    in_maps = []
    for i in range(NCORES):
        hs = [HPC * i + k for k in range(HPC)]
        in_maps.append({
            "xT": xT,
            "wq": np.ascontiguousarray(
                Wr[:, hs, 0, :].reshape(D_IN, HPC * DH)).astype(bf),
            "wk": np.ascontiguousarray(
                Wr[:, hs, 1, :].reshape(D_IN, HPC * DH)).astype(bf),
            "wv": np.ascontiguousarray(
                Wr[:, hs, 2, :].reshape(D_IN, HPC * DH)).astype(bf),
            "bq": np.ascontiguousarray(br[hs, 0, :].reshape(HPC * DH)),
            "bk": np.ascontiguousarray(br[hs, 1, :].reshape(HPC * DH)),
            "wo": np.ascontiguousarray(
                W_out[hs[0] * DH:(hs[-1] + 1) * DH, :]).astype(bf),
            "cosT": cosT,
            "sinTs": sinTs,
        })
    return in_maps


def kernel(x, W_qkv, b_qkv, W_out, b_out, mask):
    x = np.asarray(x, dtype=np.float32)
    W_qkv = np.asarray(W_qkv, dtype=np.float32)
    b_qkv = np.asarray(b_qkv, dtype=np.float32)
    W_out = np.asarray(W_out, dtype=np.float32)
    in_maps = _host_prep(x, W_qkv, b_qkv, W_out, np.asarray(mask))
    nc = _get_nc()
    res = run_bass_kernel_spmd(nc, in_maps, core_ids=list(range(NCORES)))
    out = np.asarray(res.results[0]["y"], dtype=np.float32)
    for i in range(1, NCORES):
        out += np.asarray(res.results[i]["y"], dtype=np.float32)
    # V bias folded here: softmax weights sum to 1, so the attention output
    # is (P V_nobias)/rowsum + b_v per head; b_v flows through W_out as a
    # constant row added to every token.
    bv_all = b_qkv.reshape(H, 3, DH)[:, 2, :].reshape(D_MODEL)
    b_eff = np.asarray(b_out, dtype=np.float32) + bv_all @ W_out
    out += b_eff[None, :]
    return out.reshape(B, T, D_MODEL).astype(np.float32)
